# revision 3
# baseline (speedup 1.0000x reference)
"""Trainium2 Bass kernel: segment_sum of edge features into nodes (GNN aggregation).

Computes out[n, :] = sum over edges e with receivers[e] == n of edges[e, :],
for n in [0, 100000), edges [1000000, 64] fp32 — distributed over 8 NeuronCores.

Strategy:
  - Host: value-shard edges across 8 cores by receiver range (12500 nodes/core),
    sort each core's edges by receiver, and pack them into 128-edge "chunks"
    grouped by 128-node windows (fixed 12 chunks per window, padded).
    Edge fp32 values are split into fp16 hi + fp16 lo halves (lossless to ~1e-7).
  - Device (per core): per chunk, build a one-hot [128 tokens x 128 nodes]
    matrix (VectorEngine is_equal vs an iota row; a fraction on ScalarEngine via
    Square+is_equal to balance engines), then one TensorEngine matmul per chunk
    accumulates the window's node sums into PSUM. The hi and lo halves ride one
    matmul: the output access pattern wraps both 64-column halves onto the same
    PSUM columns, and PSUM's per-element accumulate folds hi+lo for free.
    7 windows share one PSUM bank; one ScalarEngine copy flushes the group.
  - No cross-core reduction: node ranges are disjoint; host concatenates.
"""

import os

import numpy as np

N_EDGES = 1_000_000
N_NODES = 100_000
N_FEAT = 64
N_CORES = 8
NODES_PER_CORE = N_NODES // N_CORES  # 12500
WIN = 128
N_WIN = (NODES_PER_CORE + WIN - 1) // WIN  # 98
K_CHUNKS = 12  # chunks (of 128 edges) per 128-node window; capacity 1536 edges
C_CHUNKS = N_WIN * K_CHUNKS  # 1176
GROUP_W = 7  # windows per PSUM bank / DMA / flush group; 98 = 14 * 7
CAP = K_CHUNKS * WIN  # per-window edge capacity

_NC_CACHE = None
LAST_RESULT = None  # BassKernelResults of the most recent hardware run


def _build_nc():
    global _NC_CACHE
    if _NC_CACHE is not None:
        return _NC_CACHE

    import concourse.bass as bass
    import concourse.tile as tile
    from concourse import bacc, mybir

    F16 = mybir.dt.float16
    F32 = mybir.dt.float32

    nc = bacc.Bacc("TRN2", target_bir_lowering=False)
    tokens = nc.dram_tensor("tokens", [128, C_CHUNKS, 128], F16, kind="ExternalInput")
    rel = nc.dram_tensor("rel", [128, C_CHUNKS], F32, kind="ExternalInput")
    relneg = nc.dram_tensor("relneg", [128, C_CHUNKS], F32, kind="ExternalInput")
    iota = nc.dram_tensor("iota", [128, 128], F16, kind="ExternalInput")
    out = nc.dram_tensor("out", [128, N_WIN, 64], F32, kind="ExternalOutput")

    n_groups = N_WIN // GROUP_W
    n_chunks_g = GROUP_W * K_CHUNKS
    with tile.TileContext(nc) as tc:
        with (
            tc.tile_pool(name="const", bufs=1) as const,
            tc.tile_pool(name="tok", bufs=2) as tokp,
            tc.tile_pool(name="oh", bufs=6) as ohp,
            tc.tile_pool(name="sq", bufs=4) as sqp,
            tc.tile_pool(name="ps", bufs=2, space="PSUM") as psp,
            tc.tile_pool(name="stage", bufs=2) as stp,
        ):
            iota_t = const.tile([128, 128], F16)
            nc.sync.dma_start(iota_t[:], iota[:])
            rel_t = const.tile([128, C_CHUNKS], F32)
            nc.sync.dma_start(rel_t[:], rel[:])
            relneg_t = const.tile([128, C_CHUNKS], F32)
            nc.sync.dma_start(relneg_t[:], relneg[:])

            for g in range(n_groups):
                c0 = g * n_chunks_g
                tok = tokp.tile([128, n_chunks_g, 128], F16)
                nc.sync.dma_start(tok[:], tokens[:, c0 : c0 + n_chunks_g, :])
                ps = psp.tile([128, GROUP_W * 64], F32)
                for wi in range(GROUP_W):
                    for c in range(K_CHUNKS):
                        lc = wi * K_CHUNKS + c
                        gc = c0 + lc
                        oh = ohp.tile([128, 128], F16, tag="oh")
                        if lc % 8 < 3:
                            # ScalarEngine path: (iota - rel)^2 then ==0 on DVE
                            sq = sqp.tile([128, 128], F16, tag="sq")
                            nc.scalar.activation(
                                out=sq[:],
                                in_=iota_t[:],
                                func=mybir.ActivationFunctionType.Square,
                                bias=relneg_t[:, gc : gc + 1],
                            )
                            nc.vector.tensor_scalar(
                                out=oh[:], in0=sq[:], scalar1=0.0, scalar2=None,
                                op0=mybir.AluOpType.is_equal)
                        else:
                            nc.vector.tensor_scalar(
                                out=oh[:], in0=iota_t[:],
                                scalar1=rel_t[:, gc : gc + 1], scalar2=None,
                                op0=mybir.AluOpType.is_equal)
                        pslice = ps[:, wi * 64 : (wi + 1) * 64]
                        wrap = bass.AP(
                            pslice.tensor, pslice.offset,
                            [list(pslice.ap[0]), [0, 2], [1, 64]])
                        nc.tensor.matmul(
                            out=wrap, lhsT=oh[:], rhs=tok[:, lc, :],
                            start=(lc == 0), stop=(lc == n_chunks_g - 1),
                            skip_group_check=True)
                stage = stp.tile([128, GROUP_W * 64], F32)
                nc.scalar.copy(stage[:], ps[:])
                nc.sync.dma_start(out[:, g * GROUP_W : (g + 1) * GROUP_W, :], stage[:])
    nc.compile()
    _NC_CACHE = nc
    return nc


def _numpy_segment_sum(edges, receivers):
    out = np.zeros((N_NODES, N_FEAT), np.float32)
    r = np.asarray(receivers).astype(np.int64)
    ok = (r >= 0) & (r < N_NODES)
    np.add.at(out, r[ok], np.asarray(edges, np.float32)[ok])
    return out


def kernel(edges, nodes, receivers):
    global LAST_RESULT

    edges = np.ascontiguousarray(edges, dtype=np.float32)
    n_nodes = nodes.shape[0]
    r = np.asarray(receivers).astype(np.int64)
    if (
        edges.shape != (N_EDGES, N_FEAT)
        or n_nodes != N_NODES
        or r.shape != (N_EDGES,)
        or os.environ.get("KERNEL_FORCE_NUMPY")
    ):
        return _numpy_segment_sum(edges, receivers)

    # ---- host-side sharding / packing ----
    order = np.argsort(r, kind="stable")
    r_s = r[order]
    bounds = np.searchsorted(r_s, NODES_PER_CORE * np.arange(N_CORES + 1))

    hi_all = edges.astype(np.float16)
    lo_all = (edges - hi_all.astype(np.float32)).astype(np.float16)

    iota = np.broadcast_to(np.arange(128, dtype=np.float16), (128, 128)).copy()

    in_maps = []
    spill_ids = []
    for i in range(N_CORES):
        lo_b, hi_b = bounds[i], bounds[i + 1]
        idx = order[lo_b:hi_b]
        rr = r_s[lo_b:hi_b] - NODES_PER_CORE * i
        w = rr >> 7
        cnt = np.bincount(w, minlength=N_WIN)
        start = np.zeros(N_WIN, np.int64)
        np.cumsum(cnt[:-1], out=start[1:])
        rank = np.arange(len(rr)) - start[w]
        keep = rank < CAP
        if not keep.all():
            spill_ids.append(idx[~keep])
            idx, rr, w, rank = idx[keep], rr[keep], w[keep], rank[keep]

        tokens = np.zeros((128, C_CHUNKS, 128), np.float16)
        relarr = np.full((128, C_CHUNKS), -1.0, np.float32)
        gchunk = w * K_CHUNKS + (rank >> 7)
        slot = rank & 127
        tokens[slot, gchunk, 0:64] = hi_all[idx]
        tokens[slot, gchunk, 64:128] = lo_all[idx]
        relarr[slot, gchunk] = (rr & 127).astype(np.float32)
        in_maps.append(
            {"tokens": tokens, "rel": relarr, "relneg": -relarr, "iota": iota}
        )

    # ---- device run ----
    from concourse.bass_utils import run_bass_kernel_spmd

    nc = _build_nc()
    res = run_bass_kernel_spmd(nc, in_maps, core_ids=list(range(N_CORES)))
    LAST_RESULT = res

    # ---- unshard ----
    full = np.empty((N_NODES, N_FEAT), np.float32)
    for i in range(N_CORES):
        dev = res.results[i]["out"]  # [128, N_WIN, 64]
        part = dev.transpose(1, 0, 2).reshape(N_WIN * 128, 64)[:NODES_PER_CORE]
        full[i * NODES_PER_CORE : (i + 1) * NODES_PER_CORE] = part

    if spill_ids:
        sp = np.concatenate(spill_ids)
        np.add.at(full, r[sp], edges[sp])

    return full


# revision 4
# speedup vs baseline: 1.2659x; 1.2659x over previous
"""Trainium2 Bass kernel: segment_sum of edge features into nodes (GNN aggregation).

Computes out[n, :] = sum over edges e with receivers[e] == n of edges[e, :],
for n in [0, 100000), edges [1000000, 64] fp32 — distributed over 8 NeuronCores
(value-sharded by receiver range, 12500 nodes per core; node ranges are
disjoint so no cross-core reduction is needed).

Device algorithm ("degree-slotted static-ones matmul"):
  - Host packs each node's edges (sorted by receiver) into "node-rows" of 4
    slots; a chunk = 32 node-rows = 128 slots = one TensorEngine matmul.
  - The stationary operand is a single STATIC block-ones matrix [128, 32]
    (ones[s, j] = 1 iff s//4 == j): out[j, :] = sum of the 4 slots of row j.
    No per-chunk weight generation at all (no one-hot, zero VectorEngine work).
  - Edge fp32 values ride as fp16 hi + fp16 lo halves in one matmul: the
    output access pattern wraps both 64-column halves onto the same PSUM
    columns and PSUM's per-element accumulate folds hi+lo (error ~1e-7).
  - Four chunks pack one 128-partition PSUM block via column tiling
    (tile_position=(0, 32b)); 7 blocks share a PSUM bank; one ScalarEngine
    copy flushes the bank, then a contiguous DMA writes the rows out.
  - Host folds the ~3 rows per node with one np.add.reduceat.
"""

import os

import numpy as np

N_EDGES = 1_000_000
N_NODES = 100_000
N_FEAT = 64
N_CORES = 8
NODES_PER_CORE = N_NODES // N_CORES  # 12500
S = 4  # slots per node-row
BLK_W = 7  # 128-row blocks per PSUM bank / flush group
N_BLOCKS = 294  # row capacity 294*128 = 37632 (mean demand ~36625)
ROWS_CAP = N_BLOCKS * 128
C_CHUNKS = N_BLOCKS * 4  # 1176 chunks of 128 slots
N_GROUPS = N_BLOCKS // BLK_W  # 42

_NC_CACHE = None
LAST_RESULT = None  # BassKernelResults of the most recent hardware run


def _build_nc():
    global _NC_CACHE
    if _NC_CACHE is not None:
        return _NC_CACHE

    import concourse.bass as bass
    import concourse.tile as tile
    from concourse import bacc, mybir

    F16 = mybir.dt.float16
    F32 = mybir.dt.float32

    nc = bacc.Bacc("TRN2", target_bir_lowering=False)
    tokens = nc.dram_tensor("tokens", [128, C_CHUNKS, 128], F16, kind="ExternalInput")
    ones = nc.dram_tensor("ones", [128, 32], F16, kind="ExternalInput")
    out = nc.dram_tensor("out", [128, N_BLOCKS, 64], F32, kind="ExternalOutput")

    with tile.TileContext(nc) as tc:
        with (
            tc.tile_pool(name="const", bufs=1) as const,
            tc.tile_pool(name="tok", bufs=3) as tokp,
            tc.tile_pool(name="ps", bufs=2, space="PSUM") as psp,
            tc.tile_pool(name="stage", bufs=2) as stp,
        ):
            ones_t = const.tile([128, 32], F16)
            nc.sync.dma_start(ones_t[:], ones[:])

            for g in range(N_GROUPS):
                c0 = g * BLK_W * 4
                tok = tokp.tile([128, BLK_W * 4, 128], F16)
                nc.sync.dma_start(tok[:], tokens[:, c0 : c0 + BLK_W * 4, :])
                ps = psp.tile([128, BLK_W * 64], F32)
                for blk in range(BLK_W):
                    for b in range(4):
                        lc = blk * 4 + b
                        pslice = ps[32 * b : 32 * b + 32,
                                    blk * 64 : (blk + 1) * 64]
                        o = bass.AP(pslice.tensor, pslice.offset,
                                    [list(pslice.ap[0]), [0, 2], [1, 64]])
                        nc.tensor.matmul(
                            out=o, lhsT=ones_t[:], rhs=tok[:, lc, :],
                            start=True, stop=True, tile_position=(0, 32 * b))
                stage = stp.tile([128, BLK_W * 64], F32)
                nc.scalar.copy(stage[:], ps[:])
                nc.sync.dma_start(out[:, g * BLK_W : (g + 1) * BLK_W, :], stage[:])
    nc.compile()
    _NC_CACHE = nc
    return nc


def _numpy_segment_sum(edges, receivers):
    out = np.zeros((N_NODES, N_FEAT), np.float32)
    r = np.asarray(receivers).astype(np.int64)
    ok = (r >= 0) & (r < N_NODES)
    np.add.at(out, r[ok], np.asarray(edges, np.float32)[ok])
    return out


def kernel(edges, nodes, receivers):
    global LAST_RESULT

    edges = np.ascontiguousarray(edges, dtype=np.float32)
    n_nodes = nodes.shape[0]
    r = np.asarray(receivers).astype(np.int64)
    if (
        edges.shape != (N_EDGES, N_FEAT)
        or n_nodes != N_NODES
        or r.shape != (N_EDGES,)
        or os.environ.get("KERNEL_FORCE_NUMPY")
    ):
        return _numpy_segment_sum(edges, receivers)

    # ---- host-side sharding / packing ----
    order = np.argsort(r, kind="stable")
    r_s = r[order]
    bounds = np.searchsorted(r_s, NODES_PER_CORE * np.arange(N_CORES + 1))

    hi_all = edges.astype(np.float16)
    lo_all = (edges - hi_all.astype(np.float32)).astype(np.float16)

    ones = np.zeros((128, 32), np.float16)
    ones[np.arange(128), np.arange(128) // S] = 1.0

    in_maps = []
    spill_ids = []
    meta = []
    for i in range(N_CORES):
        lo_b, hi_b = bounds[i], bounds[i + 1]
        idx = order[lo_b:hi_b]
        rr = r_s[lo_b:hi_b] - NODES_PER_CORE * i

        d = np.bincount(rr, minlength=NODES_PER_CORE)
        rows_n = (d + S - 1) // S
        total = rows_n.sum()
        if total > ROWS_CAP:
            # Spill whole tail nodes to a host-side fixup (pathological skew).
            cut = int(np.searchsorted(np.cumsum(rows_n), ROWS_CAP, side="right"))
            sp = rr >= cut
            spill_ids.append(idx[sp])
            idx, rr = idx[~sp], rr[~sp]
            d = np.bincount(rr, minlength=NODES_PER_CORE)
            rows_n = (d + S - 1) // S

        rowstart_n = np.zeros(NODES_PER_CORE, np.int64)
        np.cumsum(rows_n[:-1], out=rowstart_n[1:])
        node_first = np.zeros(NODES_PER_CORE, np.int64)
        np.cumsum(d[:-1], out=node_first[1:])

        rank = np.arange(len(rr)) - node_first[rr]
        row_e = rowstart_n[rr] + (rank >> 2)
        slot_e = rank & (S - 1)

        tokens = np.zeros((128, C_CHUNKS, 128), np.float16)
        lc = row_e >> 5
        p = (row_e & 31) * S + slot_e
        tokens[p, lc, 0:64] = hi_all[idx]
        tokens[p, lc, 64:128] = lo_all[idx]
        in_maps.append({"tokens": tokens, "ones": ones})
        meta.append((rows_n, rowstart_n))

    # ---- device run ----
    from concourse.bass_utils import run_bass_kernel_spmd

    nc = _build_nc()
    res = run_bass_kernel_spmd(nc, in_maps, core_ids=list(range(N_CORES)))
    LAST_RESULT = res

    # ---- unshard: fold node-rows back into nodes ----
    full = np.zeros((N_NODES, N_FEAT), np.float32)
    for i in range(N_CORES):
        dev = res.results[i]["out"]  # [128, N_BLOCKS, 64]
        rows_arr = dev.transpose(1, 0, 2).reshape(ROWS_CAP, 64)
        rows_n, rowstart_n = meta[i]
        nz = rows_n > 0
        if nz.any():
            seg = np.add.reduceat(rows_arr, rowstart_n[nz])
            block = full[i * NODES_PER_CORE : (i + 1) * NODES_PER_CORE]
            block[nz] = seg

    if spill_ids:
        sp = np.concatenate(spill_ids)
        np.add.at(full, r[sp], edges[sp])

    return full
